# revision 1
# baseline (speedup 1.0000x reference)
"""Causal dilated 1D conv (B=16, C=32, L=131072, KW=3, dil=4, left-pad 8)
as a Bass/Tile kernel on 8 Trainium2 NeuronCores.

Strategy
--------
Data-parallel: batch dim 16 -> 2 batches per core; weights replicated.

Per core the conv is computed as 3 shifted matmuls accumulated in PSUM:
    out[:, t] = sum_k  W_k @ x[:, t - 8 + 4k],   W_k in R^{32x32}.

With only 32 channels a plain matmul would use 32 of 128 partitions.  We
instead pack 4 consecutive column chunks of the sequence into 4 partition
groups (32 channels each) and make the stationary operand a 128x128
block-diagonal replication of W_k^T (built on the host), so one matmul
processes 4 chunks at once; the three tap matmuls accumulate in PSUM.

Partition p holds (channel c = p//4, group g = p%4) — channel-major so the
outermost DMA access-pattern dim is 32 wide, which the hardware DGE splits
across all 16 SDMA engines (group-major only got 4 engines = ~100 GB/s).

x is left-padded with PAD zero columns on the host, so every tile's halo
load is uniform (no edge case).  x / W / out are cast to fp16 on the host
(halves DMA traffic on both streams; PE runs fp16 at full rate) while the
PSUM accumulation stays fp32; measured rel err vs the fp32 reference is
~3.6e-4.  Tile widths taper at the start/end for faster pipeline ramp/drain.
"""

import numpy as np

import concourse.bass as bass
import concourse.mybir as mybir
from concourse.tile import TileContext
from concourse.bass_utils import run_bass_kernel_spmd

# Problem constants (hardcoded per harness contract).
B, C, L = 16, 32, 131072
KW, DIL, PAD = 3, 4, 8

N_CORES = 8
B_PER_CORE = B // N_CORES          # 2
GROUPS = 4                         # partition groups of 32 channels
GTILE = 4096                       # columns per group per tile
HALO = PAD                         # left halo = (KW-1)*DIL
LP = L + PAD                       # padded row length in DRAM
MM_N = 512                         # matmul moving dim (one PSUM bank, fp32)
NSUB = GTILE // MM_N               # psum tiles per group-chunk
PSUM_CHUNK = 8                     # psum banks usable per accumulate round

DT_MM = mybir.dt.float16           # matmul input dtype (fp32 PSUM accumulate)
DT_OUT = mybir.dt.float16          # device-side output dtype (host upcasts)
NP_MM = np.float16


def _split_sync_waits(nc: bass.Bass, max_waits: int = 1) -> None:
    """The walrus build in this container rejects >`max_waits` sync-waits on
    an instruction.  Hoist excess waits onto fresh NoOp instructions inserted
    just before the offender on the same engine — program order on one engine
    serializes them, so semantics are unchanged."""
    ctr = 0
    for f in nc.m.functions:
        for bb in f.blocks:
            insts = bb.instructions
            new = []
            for inst in insts:
                si = getattr(inst, "sync_info", None)
                if si is not None and si.on_wait and len(si.on_wait) > max_waits:
                    waits = list(si.on_wait)
                    head, keep = waits[:-max_waits], waits[-max_waits:]
                    for w in head:
                        nop = mybir.InstNoOp(
                            name=f"splitw-{ctr}",
                            engine=inst.engine,
                            bass_nofuse=True,
                            sync_info=mybir.SyncInfo(on_wait=[w], on_update=[]),
                        )
                        ctr += 1
                        new.append(nop)
                    inst.sync_info = mybir.SyncInfo(
                        on_wait=keep, on_update=list(si.on_update or [])
                    )
                new.append(inst)
            insts[:] = new


def _build_nc() -> bass.Bass:
    nc = bass.Bass(target_bir_lowering=False, trn_type="TRN2")
    ROWS = B_PER_CORE * C  # 64 DRAM rows per core
    x = nc.dram_tensor("x", [ROWS, LP], DT_MM, kind="ExternalInput")
    w = nc.dram_tensor("w", [128, KW, 128], DT_MM, kind="ExternalInput")
    out = nc.dram_tensor("out", [ROWS, L], DT_OUT, kind="ExternalOutput")

    with TileContext(nc) as tc:
        with (
            tc.tile_pool(name="wpool", bufs=1) as wpool,
            tc.tile_pool(name="xpool", bufs=3) as xpool,
            tc.tile_pool(name="opool", bufs=3) as opool,
            tc.tile_pool(name="psum", bufs=1, space="PSUM") as psum,
        ):
            wt = wpool.tile([128, KW, 128], DT_MM)
            nc.sync.dma_start(out=wt[:], in_=w[:])

            COLS = L // GROUPS      # columns per group per batch
            for b in range(B_PER_CORE):
                base = b * C * LP       # element offset into padded x slab
                obase = b * C * L       # element offset into out slab
                # taper: small first tile (fast ramp) / small last (fast drain)
                if b == 0:
                    widths = [1024, 3072] + [GTILE] * ((COLS - 4096) // GTILE)
                else:
                    widths = [GTILE] * ((COLS - 4096) // GTILE) + [3072, 1024]
                assert sum(widths) == COLS
                g0 = 0
                for gtile in widths:
                    t0 = g0 * GROUPS
                    g0 += gtile
                    nsub = gtile // MM_N
                    xt = xpool.tile([128, HALO + GTILE], DT_MM, name="xt")
                    # partition p = c*4 + g reads x_pad[c, t0 + g*GTILE + v],
                    # v in [0, HALO+GTILE) — t0 is already halo-shifted by the
                    # host-side PAD zeros at the row start.
                    nc.sync.dma_start(
                        out=xt[:, : HALO + gtile],
                        in_=bass.AP(
                            x,
                            base + t0,
                            [[LP, C], [gtile, GROUPS], [1, HALO + gtile]],
                        ),
                    )

                    ot = opool.tile([128, GTILE], DT_OUT, name="ot")
                    for jc in range(0, nsub, PSUM_CHUNK):
                        jn = min(PSUM_CHUNK, nsub - jc)
                        pts = [
                            psum.tile([128, MM_N], mybir.dt.float32, name=f"pt{jj}")
                            for jj in range(jn)
                        ]
                        for jj in range(jn):
                            j = jc + jj
                            for k in range(KW):
                                off = j * MM_N + k * DIL
                                nc.tensor.matmul(
                                    out=pts[jj][:],
                                    lhsT=wt[:, k, :],
                                    rhs=xt[:, off : off + MM_N],
                                    start=(k == 0),
                                    stop=(k == KW - 1),
                                )
                            nc.vector.tensor_copy(
                                out=ot[:, j * MM_N : (j + 1) * MM_N], in_=pts[jj][:]
                            )

                    nc.scalar.dma_start(
                        out=bass.AP(
                            out, obase + t0, [[L, C], [gtile, GROUPS], [1, gtile]]
                        ),
                        in_=ot[:, :gtile],
                    )
    _split_sync_waits(nc)
    return nc


_NC_CACHE = None


def _get_nc() -> bass.Bass:
    global _NC_CACHE
    if _NC_CACHE is None:
        _NC_CACHE = _build_nc()
    return _NC_CACHE


def kernel(x: np.ndarray, W: np.ndarray, _trace: bool = False):
    x = np.ascontiguousarray(x, dtype=np.float32)   # (16, 32, 131072)
    W = np.ascontiguousarray(W, dtype=np.float32)   # (32, 96)

    # Left-pad with the causal zeros so the device sees a uniform halo.
    x_pad = np.zeros((B, C, LP), dtype=NP_MM)
    x_pad[:, :, PAD:] = x.astype(NP_MM)

    # Stationary weights for partition layout p = c*4 + g:
    #   w_bd[ci*4+g, k, co*4+g] = W[co, 3*ci+k]
    Wk = W.reshape(C, C, KW)                        # (co, ci, k)
    blk = Wk.transpose(1, 2, 0)                     # (ci, k, co)
    w_bd = np.zeros((128, KW, 128), dtype=NP_MM)
    view = w_bd.reshape(C, GROUPS, KW, C, GROUPS)
    for g in range(GROUPS):
        view[:, g, :, :, g] = blk

    nc = _get_nc()
    in_maps = []
    for core in range(N_CORES):
        xs = x_pad[core * B_PER_CORE : (core + 1) * B_PER_CORE].reshape(
            B_PER_CORE * C, LP
        )
        in_maps.append({"x": np.ascontiguousarray(xs), "w": w_bd})

    res = run_bass_kernel_spmd(
        nc, in_maps, core_ids=list(range(N_CORES)), trace=_trace
    )

    out = np.empty((B, C, L), dtype=np.float32)
    for core in range(N_CORES):
        out[core * B_PER_CORE : (core + 1) * B_PER_CORE] = res.results[core][
            "out"
        ].reshape(B_PER_CORE, C, L).astype(np.float32)
    if _trace:
        return out, res
    return out



# revision 2
# speedup vs baseline: 1.0436x; 1.0436x over previous
"""Causal dilated 1D conv (B=16, C=32, L=131072, KW=3, dil=4, left-pad 8)
as a Bass/Tile kernel on 8 Trainium2 NeuronCores.

Strategy
--------
Data-parallel: batch dim 16 -> 2 batches per core; weights replicated.

Phase-domain packing: with dilation 4, decompose t = 4j + s.  For each
phase s the conv is a *dense* causal kw=3 conv on the subsampled
sequence x_s[j] = x[:, 4j+s]:

    y_s[j] = sum_k W_k x_s[j-2+k]

Blocking j = 4m + r (r = 0..3) and packing partition p = 32 r + c gives

    Y[m] = A @ X[m] + B @ X[m-1]

with A, B two 128x128 host-built matrices (A holds the 9 intra-block
(r_out, r_in) tap blocks, B the 3 cross-block ones).  Two 512-col
matmuls per PSUM bank replace the baseline's three: tensor time drops
by a third, and PE utilisation rises to 37.5% density.

The host pre-permutes x into exactly the SBUF tile layout (one
contiguous [128, 1+w] fp16 blob per tile, leading halo column = block
m0-1, zeros at each phase row start), so every DMA both ways is a
single maximal-length contiguous stream -- the kernel is DMA-bound at
~420 GB/s aggregate (16 SDMA engines x ~26 GB/s).

PSUM->SBUF fp32->fp16 casts alternate between the vector and scalar
(activation) engines, each copying a 4-bank [128, 2048] PSUM half in
one instruction; output DMA is issued from gpsimd so neither cast
engine stalls on descriptor generation.  First/last phase rows use
tapered tile widths for faster pipeline ramp/drain.
"""

import numpy as np

import concourse.bass as bass
import concourse.mybir as mybir
from concourse.tile import TileContext
from concourse.bass_utils import run_bass_kernel_spmd

# Problem constants (hardcoded per harness contract).
B, C, L = 16, 32, 131072
KW, DIL, PAD = 3, 4, 8

N_CORES = 8
B_PER_CORE = B // N_CORES          # 2
R = 4                              # sub-times per block (partition groups)
S = DIL                            # phases
M = L // (R * S)                   # 8192 block-columns per phase row
ROWS = B_PER_CORE * S              # 8 phase rows per core
GTILE = 4096                       # block-columns per x tile
HALF = 2048                        # psum half width (4 fp32 banks)
MM_N = 512                         # one PSUM bank of fp32

DT = mybir.dt.float16
NPDT = np.float16


def _row_widths(row: int) -> list[int]:
    # taper: small first tiles (fast ramp) / small last (fast drain)
    if row == 0:
        return [1024, 3072, GTILE]
    if row == ROWS - 1:
        return [GTILE, 3072, 1024]
    return [GTILE, GTILE]


def _tiles():
    """(row, col0, width, x_elem_offset, out_elem_offset) per tile."""
    xo = oo = 0
    out = []
    for row in range(ROWS):
        c0 = 0
        for w in _row_widths(row):
            out.append((row, c0, w, xo, oo))
            xo += 128 * (1 + w)
            oo += 128 * w
            c0 += w
    return out, xo, oo


TILES, TOT_X, TOT_O = _tiles()


def _split_sync_waits(nc: bass.Bass, max_waits: int = 1) -> None:
    """The walrus build in this container rejects >`max_waits` sync-waits on
    an instruction.  Hoist excess waits onto fresh NoOp instructions inserted
    just before the offender on the same engine -- program order on one engine
    serializes them, so semantics are unchanged."""
    ctr = 0
    for f in nc.m.functions:
        for bb in f.blocks:
            insts = bb.instructions
            new = []
            for inst in insts:
                si = getattr(inst, "sync_info", None)
                if si is not None and si.on_wait and len(si.on_wait) > max_waits:
                    waits = list(si.on_wait)
                    head, keep = waits[:-max_waits], waits[-max_waits:]
                    for w in head:
                        nop = mybir.InstNoOp(
                            name=f"splitw-{ctr}",
                            engine=inst.engine,
                            bass_nofuse=True,
                            sync_info=mybir.SyncInfo(on_wait=[w], on_update=[]),
                        )
                        ctr += 1
                        new.append(nop)
                    inst.sync_info = mybir.SyncInfo(
                        on_wait=keep, on_update=list(si.on_update or [])
                    )
                new.append(inst)
            insts[:] = new


def _build_nc() -> bass.Bass:
    nc = bass.Bass(target_bir_lowering=False, trn_type="TRN2")
    x = nc.dram_tensor("x", [TOT_X], DT, kind="ExternalInput")
    w = nc.dram_tensor("w", [128, 2, 128], DT, kind="ExternalInput")
    out = nc.dram_tensor("out", [TOT_O], DT, kind="ExternalOutput")

    with TileContext(nc) as tc:
        with (
            tc.tile_pool(name="wpool", bufs=1) as wpool,
            tc.tile_pool(name="xpool", bufs=3) as xpool,
            tc.tile_pool(name="opool", bufs=3) as opool,
            tc.tile_pool(name="psum", bufs=2, space="PSUM") as psum,
        ):
            wt = wpool.tile([128, 2, 128], DT)
            nc.sync.dma_start(out=wt[:], in_=w[:])

            cast_tgl = 0
            for _row, _c0, gw, xoff, ooff in TILES:
                xt = xpool.tile([128, 1 + GTILE], DT, name="xt")
                # contiguous blob: partition p's row is 1+gw fp16 values;
                # col 0 is the B-matmul halo (block m0-1, zeros at row start)
                nc.sync.dma_start(
                    out=xt[:, : 1 + gw],
                    in_=bass.AP(x, xoff, [[1 + gw, 128], [1, 1 + gw]]),
                )

                ot = opool.tile([128, GTILE], DT, name="ot")
                pos = 0
                while pos < gw:
                    half = min(HALF, gw - pos)
                    pt = psum.tile([128, HALF], mybir.dt.float32, name="pt")
                    for j in range(half // MM_N):
                        a0 = pos + j * MM_N
                        po = pt[:, j * MM_N : (j + 1) * MM_N]
                        nc.tensor.matmul(
                            out=po,
                            lhsT=wt[:, 0, :],
                            rhs=xt[:, 1 + a0 : 1 + a0 + MM_N],
                            start=True,
                            stop=False,
                        )
                        nc.tensor.matmul(
                            out=po,
                            lhsT=wt[:, 1, :],
                            rhs=xt[:, a0 : a0 + MM_N],
                            start=False,
                            stop=True,
                        )
                    dst = ot[:, pos : pos + half]
                    if cast_tgl == 0:
                        nc.vector.tensor_copy(out=dst, in_=pt[:, :half])
                    else:
                        nc.scalar.copy(out=dst, in_=pt[:, :half])
                    cast_tgl ^= 1
                    pos += half

                nc.gpsimd.dma_start(
                    out=bass.AP(out, ooff, [[gw, 128], [1, gw]]),
                    in_=ot[:, :gw],
                )
    _split_sync_waits(nc)
    return nc


_NC_CACHE = None


def _get_nc() -> bass.Bass:
    global _NC_CACHE
    if _NC_CACHE is None:
        _NC_CACHE = _build_nc()
    return _NC_CACHE


def _build_weights(W: np.ndarray) -> np.ndarray:
    """lhsT pair [p_in, {A,B}, p_out] for the phase-block scheme."""
    Wk = W.reshape(C, C, KW)  # (co, ci, k)
    A = np.zeros((128, 128), np.float32)
    Bm = np.zeros((128, 128), np.float32)
    for r_o in range(R):
        for r_i in range(R):
            k = r_i + 2 - r_o
            if 0 <= k < KW:
                A[r_o * C : (r_o + 1) * C, r_i * C : (r_i + 1) * C] = Wk[:, :, k]
            k = r_i - 2 - r_o
            if 0 <= k < KW:
                Bm[r_o * C : (r_o + 1) * C, r_i * C : (r_i + 1) * C] = Wk[:, :, k]
    return np.ascontiguousarray(
        np.stack([A.T, Bm.T], axis=1).astype(NPDT)  # [p_in, 2, p_out]
    )


def kernel(x: np.ndarray, W: np.ndarray, _trace: bool = False):
    x = np.ascontiguousarray(x, dtype=np.float32)   # (16, 32, 131072)
    W = np.ascontiguousarray(W, dtype=np.float32)   # (32, 96)

    wab = _build_weights(W)

    # X_dev[core][row = b*S + s, p = 32 r + c, m] = x[b, c, 16 m + 4 r + s]
    x16 = x.astype(NPDT).reshape(N_CORES, B_PER_CORE, C, M, R, S)
    xdev = np.ascontiguousarray(
        x16.transpose(0, 1, 5, 4, 2, 3).reshape(N_CORES, ROWS, 128, M)
    )

    nc = _get_nc()
    in_maps = []
    for core in range(N_CORES):
        blob = np.empty(TOT_X, dtype=NPDT)
        for row, c0, gw, xoff, _oo in TILES:
            t = blob[xoff : xoff + 128 * (1 + gw)].reshape(128, 1 + gw)
            t[:, 1:] = xdev[core, row, :, c0 : c0 + gw]
            t[:, 0] = 0.0 if c0 == 0 else xdev[core, row, :, c0 - 1]
        in_maps.append({"x": blob, "w": wab})

    res = run_bass_kernel_spmd(
        nc, in_maps, core_ids=list(range(N_CORES)), trace=_trace
    )

    odev = np.empty((N_CORES, ROWS, 128, M), dtype=NPDT)
    for core in range(N_CORES):
        o = res.results[core]["out"]
        for row, c0, gw, _xo, ooff in TILES:
            odev[core, row, :, c0 : c0 + gw] = o[
                ooff : ooff + 128 * gw
            ].reshape(128, gw)

    # out[b, c, 16 m + 4 r + s] = odev[row = b*S+s, 32 r + c, m]
    out = np.ascontiguousarray(
        odev.reshape(N_CORES, B_PER_CORE, S, R, C, M)
        .transpose(0, 1, 4, 5, 3, 2)
        .reshape(B, C, L)
        .astype(np.float32)
    )
    if _trace:
        return out, res
    return out


# revision 3
# speedup vs baseline: 1.0783x; 1.0333x over previous
"""Causal dilated 1D conv (B=16, C=32, L=131072, KW=3, dil=4, left-pad 8)
as a Bass/Tile kernel on 8 Trainium2 NeuronCores.

Strategy
--------
Data-parallel: batch dim 16 -> 2 batches per core; weights replicated.

Phase-domain packing: with dilation 4, decompose t = 4j + s.  For each
phase s the conv is a *dense* causal kw=3 conv on the subsampled
sequence x_s[j] = x[:, 4j+s]:

    y_s[j] = sum_k W_k x_s[j-2+k]

Blocking j = 4m + r (r = 0..3) and packing partition p = 32 r + c gives

    Y[m] = A @ X[m] + B @ X[m-1]

with A, B two 128x128 host-built matrices (A holds the 9 intra-block
(r_out, r_in) tap blocks, B the 3 cross-block ones).  Two 512-col
matmuls per PSUM bank replace a naive scheme's three; PE density 37.5%.

The host pre-permutes x into exactly the SBUF tile layout (one
contiguous [128, w] fp16 blob per tile), so every DMA both ways is a
maximal contiguous stream of 8192-byte 512B-aligned runs -- the kernel
is DMA-bound at ~420 GB/s aggregate (16 SDMA engines x ~26 GB/s).
The one-column halo the B matmul needs at each tile seam is NOT
shipped twice: the seam PSUM column is patched with a 1-wide matmul
against the previous tile's last column (still resident in SBUF).

PSUM->SBUF fp32->fp16 casts alternate between the vector and scalar
(activation) engines, each draining a 4-bank [128, 2048] PSUM half in
one instruction; output DMA is issued from the scalar engine's queue.
First/last phase rows use tapered tile widths for faster ramp/drain.
"""

import numpy as np

import concourse.bass as bass
import concourse.mybir as mybir
from concourse.tile import TileContext
from concourse.bass_utils import run_bass_kernel_spmd

# Problem constants (hardcoded per harness contract).
B, C, L = 16, 32, 131072
KW, DIL, PAD = 3, 4, 8

N_CORES = 8
B_PER_CORE = B // N_CORES          # 2
R = 4                              # sub-times per block (partition groups)
S = DIL                            # phases
M = L // (R * S)                   # 8192 block-columns per phase row
ROWS = B_PER_CORE * S              # 8 phase rows per core
GTILE = 4096                       # block-columns per x tile
HALF = 2048                        # psum half width (4 fp32 banks)
MM_N = 512                         # one PSUM bank of fp32

DT = mybir.dt.float16
NPDT = np.float16


def _row_widths(row: int) -> list[int]:
    # taper: small first tiles (fast ramp) / small last (fast drain)
    if row == 0:
        return [1024, 3072, GTILE]
    if row == ROWS - 1:
        return [GTILE, 3072, 1024]
    return [GTILE, GTILE]


def _tiles():
    """(row, col0, width, elem_offset) per tile; x and out share layout."""
    off = 0
    out = []
    for row in range(ROWS):
        c0 = 0
        for w in _row_widths(row):
            out.append((row, c0, w, off))
            off += 128 * w
            c0 += w
    return out, off


TILES, TOT = _tiles()


def _split_sync_waits(nc: bass.Bass, max_waits: int = 1) -> None:
    """The walrus build in this container rejects >`max_waits` sync-waits on
    an instruction.  Hoist excess waits onto fresh NoOp instructions inserted
    just before the offender on the same engine -- program order on one engine
    serializes them, so semantics are unchanged."""
    ctr = 0
    for f in nc.m.functions:
        for bb in f.blocks:
            insts = bb.instructions
            new = []
            for inst in insts:
                si = getattr(inst, "sync_info", None)
                if si is not None and si.on_wait and len(si.on_wait) > max_waits:
                    waits = list(si.on_wait)
                    head, keep = waits[:-max_waits], waits[-max_waits:]
                    for w in head:
                        nop = mybir.InstNoOp(
                            name=f"splitw-{ctr}",
                            engine=inst.engine,
                            bass_nofuse=True,
                            sync_info=mybir.SyncInfo(on_wait=[w], on_update=[]),
                        )
                        ctr += 1
                        new.append(nop)
                    inst.sync_info = mybir.SyncInfo(
                        on_wait=keep, on_update=list(si.on_update or [])
                    )
                new.append(inst)
            insts[:] = new


def _build_nc() -> bass.Bass:
    nc = bass.Bass(target_bir_lowering=False, trn_type="TRN2")
    x = nc.dram_tensor("x", [TOT], DT, kind="ExternalInput")
    w = nc.dram_tensor("w", [128, 2, 128], DT, kind="ExternalInput")
    out = nc.dram_tensor("out", [TOT], DT, kind="ExternalOutput")

    with TileContext(nc) as tc:
        with (
            tc.tile_pool(name="wpool", bufs=1) as wpool,
            tc.tile_pool(name="xpool", bufs=4) as xpool,
            tc.tile_pool(name="opool", bufs=4) as opool,
            tc.tile_pool(name="psum", bufs=2, space="PSUM") as psum,
        ):
            wt = wpool.tile([128, 2, 128], DT)
            nc.sync.dma_start(out=wt[:], in_=w[:])

            cast_tgl = 0
            prev_xt = None
            prev_gw = 0
            for _row, c0, gw, off in TILES:
                xt = xpool.tile([128, GTILE], DT, name="xt")
                nc.sync.dma_start(
                    out=xt[:, :gw],
                    in_=bass.AP(x, off, [[gw, 128], [1, gw]]),
                )

                ot = opool.tile([128, GTILE], DT, name="ot")
                pos = 0
                while pos < gw:
                    half = min(HALF, gw - pos)
                    pt = psum.tile([128, HALF], mybir.dt.float32, name="pt")
                    for j in range(half // MM_N):
                        a0 = pos + j * MM_N
                        po = pt[:, j * MM_N : (j + 1) * MM_N]
                        nc.tensor.matmul(
                            out=po,
                            lhsT=wt[:, 0, :],
                            rhs=xt[:, a0 : a0 + MM_N],
                            start=True,
                            stop=False,
                        )
                        if a0 > 0:
                            # B matmul: rhs is the same tile shifted one
                            # block-column left.
                            nc.tensor.matmul(
                                out=po,
                                lhsT=wt[:, 1, :],
                                rhs=xt[:, a0 - 1 : a0 - 1 + MM_N],
                                start=False,
                                stop=True,
                            )
                        else:
                            # Tile seam: column 0's B operand is the previous
                            # tile's last column (zero at a phase-row start).
                            nc.tensor.matmul(
                                out=pt[:, 1:MM_N],
                                lhsT=wt[:, 1, :],
                                rhs=xt[:, 0 : MM_N - 1],
                                start=False,
                                stop=(c0 == 0),
                                skip_group_check=True,
                            )
                            if c0 > 0:
                                nc.tensor.matmul(
                                    out=pt[:, 0:1],
                                    lhsT=wt[:, 1, :],
                                    rhs=prev_xt[:, prev_gw - 1 : prev_gw],
                                    start=False,
                                    stop=True,
                                    skip_group_check=True,
                                )
                    dst = ot[:, pos : pos + half]
                    if cast_tgl == 0:
                        nc.vector.tensor_copy(out=dst, in_=pt[:, :half])
                    else:
                        nc.scalar.copy(out=dst, in_=pt[:, :half])
                    cast_tgl ^= 1
                    pos += half

                nc.scalar.dma_start(
                    out=bass.AP(out, off, [[gw, 128], [1, gw]]),
                    in_=ot[:, :gw],
                )
                prev_xt, prev_gw = xt, gw
    _split_sync_waits(nc)
    return nc


_NC_CACHE = None


def _get_nc() -> bass.Bass:
    global _NC_CACHE
    if _NC_CACHE is None:
        _NC_CACHE = _build_nc()
    return _NC_CACHE


def _build_weights(W: np.ndarray) -> np.ndarray:
    """lhsT pair [p_in, {A,B}, p_out] for the phase-block scheme."""
    Wk = W.reshape(C, C, KW)  # (co, ci, k)
    A = np.zeros((128, 128), np.float32)
    Bm = np.zeros((128, 128), np.float32)
    for r_o in range(R):
        for r_i in range(R):
            k = r_i + 2 - r_o
            if 0 <= k < KW:
                A[r_o * C : (r_o + 1) * C, r_i * C : (r_i + 1) * C] = Wk[:, :, k]
            k = r_i - 2 - r_o
            if 0 <= k < KW:
                Bm[r_o * C : (r_o + 1) * C, r_i * C : (r_i + 1) * C] = Wk[:, :, k]
    return np.ascontiguousarray(
        np.stack([A.T, Bm.T], axis=1).astype(NPDT)  # [p_in, 2, p_out]
    )


def kernel(x: np.ndarray, W: np.ndarray, _trace: bool = False):
    x = np.ascontiguousarray(x, dtype=np.float32)   # (16, 32, 131072)
    W = np.ascontiguousarray(W, dtype=np.float32)   # (32, 96)

    wab = _build_weights(W)

    # X_dev[core][row = b*S + s, p = 32 r + c, m] = x[b, c, 16 m + 4 r + s]
    x16 = x.astype(NPDT).reshape(N_CORES, B_PER_CORE, C, M, R, S)
    xdev = np.ascontiguousarray(
        x16.transpose(0, 1, 5, 4, 2, 3).reshape(N_CORES, ROWS, 128, M)
    )

    nc = _get_nc()
    in_maps = []
    for core in range(N_CORES):
        blob = np.empty(TOT, dtype=NPDT)
        for row, c0, gw, off in TILES:
            blob[off : off + 128 * gw].reshape(128, gw)[:] = xdev[
                core, row, :, c0 : c0 + gw
            ]
        in_maps.append({"x": blob, "w": wab})

    res = run_bass_kernel_spmd(
        nc, in_maps, core_ids=list(range(N_CORES)), trace=_trace
    )

    odev = np.empty((N_CORES, ROWS, 128, M), dtype=NPDT)
    for core in range(N_CORES):
        o = res.results[core]["out"]
        for row, c0, gw, off in TILES:
            odev[core, row, :, c0 : c0 + gw] = o[off : off + 128 * gw].reshape(
                128, gw
            )

    # out[b, c, 16 m + 4 r + s] = odev[row = b*S+s, 32 r + c, m]
    out = np.ascontiguousarray(
        odev.reshape(N_CORES, B_PER_CORE, S, R, C, M)
        .transpose(0, 1, 4, 5, 3, 2)
        .reshape(B, C, L)
        .astype(np.float32)
    )
    if _trace:
        return out, res
    return out
